# revision 1
# baseline (speedup 1.0000x reference)
"""DiffAttention TRN2 kernel: 8-way (batch x seq-half) sharded, zero collectives.

Shapes: x [4, 4096, 1024], H=16 heads, head organized as (h, 2 branches, 32 dims),
v head dim 64. Each core handles one (batch, query-half): 2048 query rows,
recomputes K/V for its batch's full 4096 keys (cheap vs any collective).

Layout strategy (everything transposed so contractions sit on partitions):
  - qkv phase: Q^T [1024, 2048], K^T [1024, 4096] (c on partitions) and
    V [4096, 1024] (tokens on partitions) written to DRAM scratch.
  - attention per (head, branch): S^T tiles [128 k, 1024 q] in PSUM from
    lhsT=K_h^T slice [32, 128], rhs=Q_h^T [32, 512] (f32r full-rate matmuls);
    exp on ACT (scale=1/sqrt(hd) folded in); PV accumulates
    O^T [65, 2048] with lhsT=V_aug [128, 65] (65th col = ones -> softmax
    denominators land in row 64 free of charge).
  - DiffAttn combine + RMS norm done column-wise on O^T with row-broadcasts
    done via SBUF->SBUF DMA; RMS col-sums via ones-vector matmul.
  - proj: lhsT = o^T accumulator tile directly, bias via K=1 ones matmul.
"""

import os
import sys

import numpy as np

for p in ("/opt/trn_rl_repo",):
    if p not in sys.path:
        sys.path.insert(0, p)

import concourse.bass as bass
import concourse.bacc as bacc_mod
import concourse.mybir as mybir
from concourse.bass_utils import run_bass_kernel_spmd
from concourse.tile import TileContext

F32 = mybir.dt.float32
F32R = mybir.dt.float32r

B, N, DIM, H, HD = 4, 4096, 1024, 16, 32
VD = 2 * HD  # 64, per-head v dim
NQ = 2048  # query rows per core
NCORES = 8
LAMBDA_INIT = 0.2
EPS = 1e-5
SCALE = HD ** -0.5

_CACHE = {}


def _r(ap):
    return ap.bitcast(F32R)


def build_nc(lam: float):
    nc = bacc_mod.Bacc(None, target_bir_lowering=False)

    xbt = nc.declare_dram_parameter("xbt", [DIM, N], F32, isOutput=False)
    wqkvt = nc.declare_dram_parameter("wqkvt", [DIM, 3 * DIM], F32, isOutput=False)
    wprojt = nc.declare_dram_parameter("wprojt", [DIM, DIM], F32, isOutput=False)
    bproj = nc.declare_dram_parameter("bproj", [1, DIM], F32, isOutput=False)
    weff = nc.declare_dram_parameter("weff", [VD, 1], F32, isOutput=False)
    y = nc.declare_dram_parameter("y", [NQ, DIM], F32, isOutput=True)

    qt_s = nc.dram_tensor("qt_scratch", [DIM, NQ], F32)
    kt_s = nc.dram_tensor("kt_scratch", [DIM, N], F32)
    v_s = nc.dram_tensor("v_scratch", [N, DIM], F32)

    KT = N // 128  # 32 key tiles
    CIN = DIM // 128  # 8 contraction tiles

    with nc.allow_low_precision(reason="f32r tiles are bit-identical fp32"), \
         TileContext(nc) as tc:
        # ---------------- persistent pools ----------------
        with (
            tc.tile_pool(name="const", bufs=1) as constp,
            tc.tile_pool(name="psA", bufs=2, space="PSUM") as psA,  # [128,1024] slots
            tc.tile_pool(name="psO", bufs=1, space="PSUM") as psO,  # [65,2048] slot
        ):
            ones64 = constp.tile([VD, 1], F32R)
            nc.vector.memset(ones64.bitcast(F32), 1.0)
            ones1 = constp.tile([1, 128], F32R)
            nc.vector.memset(ones1.bitcast(F32), 1.0)
            ones_vd = constp.tile([1, VD], F32R)
            nc.vector.memset(ones_vd.bitcast(F32), 1.0)
            eps_t = constp.tile([1, 1], F32)
            nc.vector.memset(eps_t, EPS)
            weff_t = constp.tile([VD, 1], F32)
            nc.sync.dma_start(out=weff_t, in_=weff[:, :])
            # ================= phase A: qkv =================
            with (
                tc.tile_pool(name="xbt_p", bufs=1) as xbtp,
                tc.tile_pool(name="wq_p", bufs=4) as wqp,
                tc.tile_pool(name="drain_p", bufs=3) as drp,
            ):
                xb = xbtp.tile([128, CIN, N], F32R)
                nc.sync.dma_start(
                    out=xb, in_=xbt[:, :].rearrange("(t p) n -> p t n", p=128).bitcast(F32R)
                )
                # --- Q^T and K^T co-tiles ---
                for co in range(2 * CIN):  # 0..7 Q, 8..15 K
                    is_q = co < CIN
                    tok = NQ if is_q else N
                    for ch in range(tok // 1024):
                        ps = psA.tile([128, 1024], F32, tag="ps")
                        for ci in range(CIN):
                            wt = wqp.tile([128, 128], F32R, tag="w")
                            nc.sync.dma_start(
                                out=wt,
                                in_=wqkvt[ci * 128:(ci + 1) * 128, co * 128:(co + 1) * 128].bitcast(F32R),
                            )
                            for sb in range(2):
                                nc.tensor.matmul(
                                    ps[:, sb * 512:(sb + 1) * 512],
                                    _r(wt),
                                    _r(xb[:, ci, ch * 1024 + sb * 512: ch * 1024 + (sb + 1) * 512]),
                                    start=(ci == 0),
                                    stop=(ci == CIN - 1),
                                )
                        dr = drp.tile([128, 1024], F32, tag="dr")
                        nc.vector.tensor_copy(dr, ps)
                        dst = qt_s if is_q else kt_s
                        coo = co if is_q else co - CIN
                        nc.sync.dma_start(
                            out=dst[coo * 128:(coo + 1) * 128, ch * 1024:(ch + 1) * 1024],
                            in_=dr,
                        )
                # --- V (untransposed) in c-chunks of 256 ---
                with tc.tile_pool(name="wv_p", bufs=8) as wvp:
                    for cc in range(DIM // 256):
                        wv_tiles = []
                        for ci in range(CIN):
                            wv = wvp.tile([128, 256], F32R, tag="wv")
                            nc.sync.dma_start(
                                out=wv,
                                in_=wqkvt[ci * 128:(ci + 1) * 128,
                                          2 * DIM + cc * 256: 2 * DIM + (cc + 1) * 256].bitcast(F32R),
                            )
                            wv_tiles.append(wv)
                        for kt in range(KT):
                            psv = psA.tile([128, 256], F32, tag="ps")
                            for ci in range(CIN):
                                nc.tensor.matmul(
                                    psv,
                                    _r(xb[:, ci, kt * 128:(kt + 1) * 128]),
                                    _r(wv_tiles[ci]),
                                    start=(ci == 0),
                                    stop=(ci == CIN - 1),
                                )
                            drv = drp.tile([128, 256], F32, tag="dr")
                            nc.vector.tensor_copy(drv, psv)
                            nc.sync.dma_start(
                                out=v_s[kt * 128:(kt + 1) * 128, cc * 256:(cc + 1) * 256],
                                in_=drv,
                            )

            # ================= phase B: attention =================
            with tc.tile_pool(name="ot", bufs=1) as otp:
              # o^T accumulator: [128 part, 8 cin-tiles, 2048 q] = 64KB/part
              ot_acc = otp.tile([128, CIN, NQ], F32R)
              with (
                tc.tile_pool(name="qk_p", bufs=2) as qkp,
                tc.tile_pool(name="vh_p", bufs=2) as vhp,
                tc.tile_pool(name="es_p", bufs=2) as esp,
                tc.tile_pool(name="o1_p", bufs=2) as o1p,
                tc.tile_pool(name="row_p", bufs=3) as rowp,
            ):
                  for h in range(H):
                      # V_aug for this head: [128, KT, 65]; col 64 = ones
                      vh = vhp.tile([128, KT, 65], F32R, tag="vh")
                      for kt0 in range(KT):
                          nc.sync.dma_start(
                              out=vh[:, kt0, 0:VD],
                              in_=v_s[kt0 * 128:(kt0 + 1) * 128,
                                      h * VD:(h + 1) * VD].bitcast(F32R),
                          )
                      nc.vector.memset(vh[:, :, VD:65].bitcast(F32), 1.0)

                      o1s = None
                      for br in range(2):
                          r0 = h * VD + br * HD
                          # Q_h^T packed [128, NQ//512 groups? -> [32*4, ...]:
                          # pack 4 column-tiles onto partitions: [128, NQ//512, 128]
                          qh = qkp.tile([128, NQ], F32R, tag="qh")
                          kh = qkp.tile([128, N // 256, 128], F32R, tag="kh")
                          for rrp in range(2):
                              nc.sync.dma_start(
                                  out=qh[rrp * 64:rrp * 64 + HD, :],
                                  in_=qt_s[r0:r0 + HD, :].bitcast(F32R),
                              )
                          for bq in range(2):
                              nc.sync.dma_start(
                                  out=kh[bq * 64:bq * 64 + HD, :, :],
                                  in_=kt_s[r0:r0 + HD, :].rearrange(
                                      "d (g b t) -> d g b t", b=2, t=128
                                  )[:, :, bq, :].bitcast(F32R),
                              )
                          o_ps = psO.tile([65, NQ], F32, tag="o")
                          for qc in range(NQ // 1024):
                              for kt in range(KT):
                                  sps = psA.tile([128, 1024], F32, tag="ps")
                                  kb = 64 * (kt % 2)
                                  klhs = kh[kb:kb + HD, kt // 2, :]
                                  for sb in range(2):
                                      qi = 2 * qc + sb
                                      nc.tensor.matmul(
                                          sps[:, sb * 512:(sb + 1) * 512],
                                          _r(klhs),
                                          _r(qh[kb:kb + HD, qi * 512:(qi + 1) * 512]),
                                          start=True,
                                          stop=True,
                                      )
                                  es = esp.tile([128, 1024], F32R, tag="es")
                                  nc.scalar.activation(
                                      es, sps, mybir.ActivationFunctionType.Exp,
                                      scale=SCALE,
                                  )
                                  for sb in range(2):
                                      nc.tensor.matmul(
                                          o_ps[:, qc * 1024 + sb * 512: qc * 1024 + (sb + 1) * 512],
                                          _r(vh[:, kt, :]),
                                          _r(es[:, sb * 512:(sb + 1) * 512]),
                                          start=(kt == 0),
                                          stop=(kt == KT - 1),
                                      )
                          if br == 0:
                              o1s = o1p.tile([65, NQ], F32, tag="o1", bufs=3)
                              nc.vector.tensor_copy(o1s, o_ps)
                          else:
                            o2s = o1p.tile([65, NQ], F32, tag="o1", bufs=3)
                            nc.vector.tensor_copy(o2s, o_ps)
                            # t = lam * s1 / s2   [1, NQ]
                            s2r = rowp.tile([1, NQ], F32, tag="row")
                            nc.vector.reciprocal(s2r, o2s[64:65, :])
                            trow = rowp.tile([1, NQ], F32R, tag="row")
                            nc.vector.tensor_scalar_mul(trow, o1s[64:65, :], lam)
                            nc.vector.tensor_mul(trow, trow, s2r)
                            od = o1p.tile([VD, NQ], F32, tag="tmp")
                            sq = o1p.tile([VD, NQ], F32R, tag="tbc")
                            msr = rowp.tile([1, NQ], F32, tag="row")
                            for i in range(NQ // 512):
                                sl = slice(i * 512, (i + 1) * 512)
                                tps = psA.tile([VD, 512], F32, tag="ps")
                                nc.tensor.matmul(
                                    tps, _r(ones_vd), _r(trow[:, sl]),
                                    start=True, stop=True,
                                )
                                nc.vector.tensor_mul(od[:, sl], tps, o2s[0:VD, sl])
                                nc.vector.tensor_sub(od[:, sl], o1s[0:VD, sl], od[:, sl])
                                # ---- RMS norm (scale-invariance: od = s1 * o) ----
                                nc.vector.tensor_mul(sq[:, sl], od[:, sl], od[:, sl])
                                rps = psA.tile([1, 512], F32, tag="ps")
                                nc.tensor.matmul(
                                    rps, _r(ones64), _r(sq[:, sl]),
                                    start=True, stop=True,
                                )
                                nc.vector.tensor_copy(msr[:, sl], rps)
                            # ms_true = msr / (VD * s1^2); sd = sqrt(ms_true+eps)
                            s1sq = rowp.tile([1, NQ], F32, tag="row")
                            nc.vector.tensor_mul(s1sq, o1s[64:65, :], o1s[64:65, :])
                            nc.vector.reciprocal(s1sq, s1sq)
                            nc.vector.tensor_mul(msr, msr, s1sq)
                            sd = rowp.tile([1, NQ], F32, tag="row")
                            nc.scalar.activation(
                                sd, msr, mybir.ActivationFunctionType.Sqrt,
                                bias=eps_t, scale=1.0 / VD,
                            )
                            rr = rowp.tile([1, NQ], F32R, tag="row")
                            nc.vector.reciprocal(rr, sd)
                            # od = s1*o -> o_normed = od * rr / s1: fold 1/s1 in
                            s1r = rowp.tile([1, NQ], F32, tag="row")
                            nc.vector.reciprocal(s1r, o1s[64:65, :])
                            nc.vector.tensor_mul(rr, rr, s1r)
                            p0 = (h % 2) * VD
                            for i in range(NQ // 512):
                                sl = slice(i * 512, (i + 1) * 512)
                                rbs = psA.tile([VD, 512], F32, tag="ps")
                                nc.tensor.matmul(
                                    rbs, _r(ones_vd), _r(rr[:, sl]),
                                    start=True, stop=True,
                                )
                                nc.vector.tensor_mul(od[:, sl], od[:, sl], rbs)
                                nc.vector.tensor_scalar_mul(
                                    ot_acc[p0:p0 + VD, h // 2, sl], od[:, sl], weff_t,
                                )

              # ================= phase C: proj =================
              with (
                  tc.tile_pool(name="wp_p", bufs=1) as wpp,
                  tc.tile_pool(name="yd_p", bufs=3) as ydp,
              ):
                  wp = wpp.tile([128, CIN, DIM], F32R)
                  nc.sync.dma_start(
                      out=wp, in_=wprojt[:, :].rearrange("(t p) n -> p t n", p=128).bitcast(F32R)
                  )
                  bp = wpp.tile([1, DIM], F32R)
                  nc.sync.dma_start(out=bp, in_=bproj[:, :].bitcast(F32R))
                  for qt in range(NQ // 128):
                      yps = psA.tile([128, 1024], F32, tag="ps")
                      for sb in range(2):
                          for ci in range(CIN):
                              nc.tensor.matmul(
                                  yps[:, sb * 512:(sb + 1) * 512],
                                  _r(ot_acc[:, ci, qt * 128:(qt + 1) * 128]),
                                  _r(wp[:, ci, sb * 512:(sb + 1) * 512]),
                                  start=(ci == 0),
                                  stop=False,
                              )
                          nc.tensor.matmul(
                              yps[:, sb * 512:(sb + 1) * 512],
                              _r(ones1),
                              _r(bp[:, sb * 512:(sb + 1) * 512]),
                              start=False,
                              stop=True,
                          )
                      yd = ydp.tile([128, 1024], F32, tag="yd")
                      nc.vector.tensor_copy(yd, yps)
                      nc.sync.dma_start(
                          out=y[qt * 128:(qt + 1) * 128, :], in_=yd
                      )
    nc.finalize()
    return nc


def kernel(x, w_qkv, w_proj, b_proj, lambda_q1, lambda_k1, lambda_q2,
           lambda_k2, sub_norm_w):
    x = np.asarray(x, np.float32)
    lam = float(
        np.exp(np.sum(np.float64(lambda_q1) * np.float64(lambda_k1)))
        - np.exp(np.sum(np.float64(lambda_q2) * np.float64(lambda_k2)))
        + LAMBDA_INIT
    )
    wqkvt = np.ascontiguousarray(np.asarray(w_qkv, np.float32).T)
    wprojt = np.ascontiguousarray(np.asarray(w_proj, np.float32).T)
    bp = np.asarray(b_proj, np.float32).reshape(1, DIM)
    weff = (np.asarray(sub_norm_w, np.float32) * (1.0 - LAMBDA_INIT)).reshape(VD, 1)

    key = round(lam, 12)
    if key not in _CACHE:
        _CACHE[key] = build_nc(lam)
    nc = _CACHE[key]

    in_maps = []
    for c in range(NCORES):
        b, half = c // 2, c % 2
        xt = np.asarray(x[b].T)  # [DIM, N]
        if half == 1:  # query rows first
            xt = np.concatenate([xt[:, NQ:], xt[:, :NQ]], axis=1)
        in_maps.append({
            "xbt": np.ascontiguousarray(xt),
            "wqkvt": wqkvt,
            "wprojt": wprojt,
            "bproj": bp,
            "weff": weff,
        })
    res = run_bass_kernel_spmd(nc, in_maps, list(range(NCORES)))
    out = np.empty((B, N, DIM), np.float32)
    for c in range(NCORES):
        b, half = c // 2, c % 2
        out[b, half * NQ:(half + 1) * NQ, :] = res.results[c]["y"]
    return out



# revision 13
# speedup vs baseline: 1.2494x; 1.2494x over previous
"""DiffAttention TRN2 kernel v2: bf16, SBUF-resident QKV, deferred combine.

Sharding: 8 cores = (batch b in 0..3) x (query-half). Each core: 2048 query
rows, recomputes K/V for its batch's full 4096 keys. Zero collectives.

v2 changes vs v1 (6.65ms baseline):
  - all matmuls bf16 (f32r tripped a sustained PE clock throttle K=4/8;
    bf16 also makes LDWEIGHTS ~4x cheaper and halves DMA/SBUF).
  - Q^T/K^T/V live in SBUF packed layouts (no DRAM scratch roundtrip, no
    per-head reloads). The qkv co-tile drain is partition-identity into
    the packed layout: for (h,br): idx=2h+br -> slot co=idx//4, partition
    offset 32*(idx%4). Offset-96 blocks are staged to base 0 on the fly
    (matmul operand APs cannot start at partition 96 - HW fault).
  - attention inner loop: S matmul (bf16) -> exp on ACT (PSUM->SBUF bf16)
    -> PV accumulate, with V augmented by a ones column so softmax
    denominators fall out of the same matmul (row 64 of O^T).
  - deferred combine: per (h,br,qc) only O^T rows 0-63 spill to DRAM
    (bf16) and denominator rows collect in SBUF [16,2048]; after all
    heads, ONE batched row-math pass (3 reciprocals total instead of 64
    single-partition ones at 12.9us each) + per-head elementwise on
    DVE/Pool with partition_broadcast for row->64-partition broadcasts.
"""

import sys

import numpy as np

for p in ("/opt/trn_rl_repo",):
    if p not in sys.path:
        sys.path.insert(0, p)

import ml_dtypes

import concourse.bass as bass
import concourse.bacc as bacc_mod
import concourse.mybir as mybir
from concourse.bass_utils import run_bass_kernel_spmd
from concourse.tile import TileContext

F32 = mybir.dt.float32
BF16 = mybir.dt.bfloat16
NPBF = ml_dtypes.bfloat16

B, N, DIM, H, HD = 4, 4096, 1024, 16, 32
VD = 2 * HD  # 64
NQ = 2048
NCORES = 8
LAMBDA_INIT = 0.2
EPS = 1e-5
SCALE = HD ** -0.5
KT = N // 128  # 32 key tiles
CIN = DIM // 128  # 8

_CACHE = {}


def _phase_a(nc, tc, xbt, wqkvt, qpack, kpack, vpack):
    """qkv projections into SBUF packed layouts."""
    xbp = tc.alloc_tile_pool(name="xb_p", bufs=1)
    wvp = tc.alloc_tile_pool(name="wv_p", bufs=1)
    wqp = tc.alloc_tile_pool(name="wq_p", bufs=8)
    psqk = tc.alloc_tile_pool(name="psQK", bufs=4, space="PSUM")
    psv_p = tc.alloc_tile_pool(name="psV", bufs=4, space="PSUM")

    wv = wvp.tile([128, CIN, DIM], BF16, name="wv")
    nc.sync.dma_start(
        out=wv,
        in_=wqkvt[:, 2 * DIM:3 * DIM].rearrange("(t p) c -> p t c", p=128),
    )
    dr_i = 0
    for e8 in range(8):
        tok0 = e8 * 512
        xb = xbp.tile([128, CIN, 512], BF16, tag="xb", name="xb")
        nc.sync.dma_start(
            out=xb,
            in_=xbt[:, tok0:tok0 + 512].rearrange("(t p) n -> p t n", p=128),
        )
        for which in range(2):  # 0 = Q, 1 = K
            if which == 0 and e8 >= 4:
                continue
            wofs = which * DIM
            dst = qpack if which == 0 else kpack
            for co in range(CIN):
                ps = psqk.tile([128, 512], F32, tag="qk", name="psqk")
                for ci in range(CIN):
                    wt = wqp.tile([128, 128], BF16, tag="w", name="wt")
                    nc.sync.dma_start(
                        out=wt,
                        in_=wqkvt[ci * 128:(ci + 1) * 128,
                                  wofs + co * 128:wofs + (co + 1) * 128],
                    )
                    nc.tensor.matmul(
                        ps,
                        wt,
                        xb[:, ci, :],
                        start=(ci == 0),
                        stop=(ci == CIN - 1),
                    )
                dsl = dst[:, co, tok0:tok0 + 512]
                if dr_i % 2 == 0:
                    nc.vector.tensor_copy(dsl, ps)
                else:
                    nc.scalar.copy(dsl, ps)
                dr_i += 1
        for ktq in range(4):  # V: tokens on partitions
            kt = e8 * 4 + ktq
            psv = [
                psv_p.tile([128, 256], F32, tag="v", name="psv")
                for _ in range(4)
            ]
            for ci in range(CIN):
                for cc in range(4):
                    nc.tensor.matmul(
                        psv[cc],
                        xb[:, ci, ktq * 128:(ktq + 1) * 128],
                        wv[:, ci, cc * 256:(cc + 1) * 256],
                        start=(ci == 0),
                        stop=(ci == CIN - 1),
                    )
            for cc in range(4):
                dsl = vpack[:, kt, 4 * cc:4 * cc + 4, 0:VD]
                if dr_i % 2 == 0:
                    nc.vector.tensor_copy(dsl, psv[cc])
                else:
                    nc.scalar.copy(dsl, psv[cc])
                dr_i += 1
    for pool in (psv_p, psqk, wqp, wvp, xbp):
        pool.release()


def _phase_b(nc, tc, qpack, kpack, vpack, ostore, den1, den2):
    """attention: S = K^T.T Q^T -> exp -> PV (with ones column denom)."""
    psS = tc.alloc_tile_pool(name="psS", bufs=2, space="PSUM")
    psO = tc.alloc_tile_pool(name="psO", bufs=2, space="PSUM")
    esp = tc.alloc_tile_pool(name="es_p", bufs=3)
    stp = tc.alloc_tile_pool(name="st_p", bufs=2)
    strp = tc.alloc_tile_pool(name="str_p", bufs=1)
    dstp = tc.alloc_tile_pool(name="dst_p", bufs=2)

    kst = qst = None
    for h in range(H):
        # matmul operand APs cannot start at partition 96 (HW fault):
        # stage the (odd h, br=1) K/Q blocks down to base 0 via DVE.
        if h % 2 == 1:
            kst = strp.tile([HD, N], BF16, tag="kst", name="kst")
            nc.vector.tensor_copy(kst, kpack[96:128, h // 2, :])
            qst = strp.tile([HD, NQ], BF16, tag="qst", name="qst")
            nc.vector.tensor_copy(qst, qpack[96:128, h // 2, :])
        for br in range(2):
            idx = 2 * h + br
            co = idx // 4
            o32 = 32 * (idx % 4)
            stray = (o32 == 96)
            for qc in range(2):
                o_ps = psO.tile([VD + 1, 1024], F32, tag="o", name="ops")
                for kt in range(KT):
                    sps = psS.tile([128, 1024], F32, tag="s", name="sps")
                    for sb in range(2):
                        if stray:
                            klhs = kst[:, kt * 128:(kt + 1) * 128]
                            qrhs = qst[:, qc * 1024 + sb * 512:
                                       qc * 1024 + (sb + 1) * 512]
                        else:
                            klhs = kpack[o32:o32 + HD, co,
                                         kt * 128:(kt + 1) * 128]
                            qrhs = qpack[o32:o32 + HD, co,
                                         qc * 1024 + sb * 512:
                                         qc * 1024 + (sb + 1) * 512]
                        nc.tensor.matmul(
                            sps[:, sb * 512:(sb + 1) * 512],
                            klhs, qrhs, start=True, stop=True,
                        )
                    es = esp.tile([128, 1024], BF16, tag="es", name="es")
                    nc.scalar.activation(
                        es, sps, mybir.ActivationFunctionType.Exp, scale=SCALE,
                    )
                    for sb in range(2):
                        nc.tensor.matmul(
                            o_ps[:, sb * 512:(sb + 1) * 512],
                            vpack[:, kt, h, :],
                            es[:, sb * 512:(sb + 1) * 512],
                            start=(kt == 0),
                            stop=(kt == KT - 1),
                        )
                stage = stp.tile([VD, 1024], BF16, tag="st", name="stage")
                nc.vector.tensor_copy(stage, o_ps[0:VD, :])
                # engine APs need 32-aligned partition base: stage the
                # denominator row then DMA it into den[h] (DMA is free-form).
                den = den1 if br == 0 else den2
                dstage = dstp.tile([1, 1024], F32, tag="ds", name="dstage")
                nc.vector.tensor_copy(dstage, o_ps[VD:VD + 1, :])
                nc.sync.dma_start(
                    out=den[h:h + 1, qc * 1024:(qc + 1) * 1024], in_=dstage,
                )
                nc.sync.dma_start(
                    out=ostore[h, br * VD:(br + 1) * VD,
                               qc * 1024:(qc + 1) * 1024],
                    in_=stage,
                )
    for pool in (dstp, strp, stp, esp, psO, psS):
        pool.release()


def _phase_c(nc, tc, lam, ostore, den1, den2, ot_acc, ones64, eps16, weff_t):
    """deferred combine: diff-attn + RMS norm, batched row math."""
    rowp = tc.alloc_tile_pool(name="row_p", bufs=1)
    odp = tc.alloc_tile_pool(name="od_p", bufs=1)
    ldp = tc.alloc_tile_pool(name="ld_p", bufs=4)
    bcp = tc.alloc_tile_pool(name="bc_p", bufs=4)
    trp = tc.alloc_tile_pool(name="tr_p", bufs=1)
    psC = tc.alloc_tile_pool(name="psC", bufs=1, space="PSUM")

    # batched row math 1: t = lam * s1 / s2, r1 = 1/s1
    r2 = rowp.tile([H, NQ], F32, tag="rt", bufs=2, name="r2")
    nc.vector.reciprocal(r2, den2)
    t_row = rowp.tile([H, NQ], F32, tag="rt", bufs=2, name="t_row")
    nc.vector.tensor_scalar_mul(t_row, den1, lam)
    nc.vector.tensor_mul(t_row, t_row, r2)
    t_bf = rowp.tile([H, NQ], BF16, tag="tb", name="t_bf")
    nc.vector.tensor_copy(t_bf, t_row)
    r1 = rowp.tile([H, NQ], F32, tag="r1", name="r1")
    nc.vector.reciprocal(r1, den1)
    msr = rowp.tile([H, NQ], F32, tag="ms", name="msr")

    od_tiles = []
    for h in range(H):
        o1t = ldp.tile([VD, NQ], BF16, tag="ld", name="o1t")
        nc.sync.dma_start(out=o1t, in_=ostore[h, 0:VD, :])
        o2t = ldp.tile([VD, NQ], BF16, tag="ld", name="o2t")
        nc.sync.dma_start(out=o2t, in_=ostore[h, VD:2 * VD, :])
        trow = trp.tile([1, NQ], BF16, tag="tr", bufs=2, name="trow")
        nc.sync.dma_start(out=trow, in_=t_bf[h:h + 1, :])
        t_bc = bcp.tile([VD, NQ], BF16, tag="bc", bufs=2, name="t_bc")
        nc.gpsimd.partition_broadcast(t_bc, trow)
        tmp = bcp.tile([VD, NQ], BF16, tag="tmp", bufs=2, name="tmp")
        nc.vector.tensor_mul(tmp, t_bc, o2t)
        od = odp.tile([VD, NQ], BF16, tag=f"od{h}", name="od")
        nc.gpsimd.tensor_sub(od, o1t, tmp)
        od_tiles.append(od)
        sq = bcp.tile([VD, NQ], BF16, tag="sq", bufs=1, name="sq")
        nc.gpsimd.tensor_mul(sq, od, od)
        msp = psC.tile([1, NQ], F32, tag="ms", name="msp")
        for c4 in range(4):
            nc.tensor.matmul(
                msp[:, c4 * 512:(c4 + 1) * 512],
                ones64,
                sq[:, c4 * 512:(c4 + 1) * 512],
                start=True, stop=True,
            )
        mstage = trp.tile([1, NQ], F32, tag="msst", bufs=1, name="mstage")
        nc.vector.tensor_copy(mstage, msp)
        nc.sync.dma_start(out=msr[h:h + 1, :], in_=mstage)

    # batched row math 2: rr = 1/(s1*sqrt(mean(od^2)/s1^2 + eps))
    a_row = rowp.tile([H, NQ], F32, tag="rt", bufs=2, name="a_row")
    nc.vector.tensor_mul(a_row, msr, r1)
    nc.vector.tensor_mul(a_row, a_row, r1)
    sd = rowp.tile([H, NQ], F32, tag="rt", bufs=2, name="sd")
    nc.scalar.activation(
        sd, a_row, mybir.ActivationFunctionType.Sqrt,
        bias=eps16, scale=1.0 / VD,
    )
    rsd = rowp.tile([H, NQ], F32, tag="rt", bufs=2, name="rsd")
    nc.vector.reciprocal(rsd, sd)
    rr = rowp.tile([H, NQ], F32, tag="rt", bufs=2, name="rr")
    nc.vector.tensor_mul(rr, rsd, r1)
    rr_bf = rowp.tile([H, NQ], BF16, tag="rb", name="rr_bf")
    nc.vector.tensor_copy(rr_bf, rr)

    for h in range(H):
        rrow = trp.tile([1, NQ], BF16, tag="tr", bufs=2, name="rrow")
        nc.sync.dma_start(out=rrow, in_=rr_bf[h:h + 1, :])
        rr_bc = bcp.tile([VD, NQ], BF16, tag="bc", bufs=2, name="rr_bc")
        nc.gpsimd.partition_broadcast(rr_bc, rrow)
        odn = bcp.tile([VD, NQ], BF16, tag="tmp", bufs=2, name="odn")
        nc.gpsimd.tensor_mul(odn, od_tiles[h], rr_bc)
        p0 = (h % 2) * VD
        nc.vector.tensor_scalar_mul(
            ot_acc[p0:p0 + VD, h // 2, :], odn, weff_t,
        )
    for pool in (psC, trp, bcp, ldp, odp, rowp):
        pool.release()


def _phase_d(nc, tc, wprojt, bproj, ot_acc, ones1, y):
    """output projection + bias."""
    wpp = tc.alloc_tile_pool(name="wp_p", bufs=1)
    ydp = tc.alloc_tile_pool(name="yd_p", bufs=3)
    psD = tc.alloc_tile_pool(name="psD", bufs=2, space="PSUM")

    wp = wpp.tile([128, CIN, DIM], BF16, name="wp")
    nc.sync.dma_start(
        out=wp, in_=wprojt[:, :].rearrange("(t p) n -> p t n", p=128),
    )
    bp = wpp.tile([1, DIM], BF16, name="bp")
    nc.sync.dma_start(out=bp, in_=bproj[:, :])
    for qt in range(NQ // 128):
        yps = psD.tile([128, 1024], F32, tag="y", name="yps")
        for sb in range(2):
            for ci in range(CIN):
                nc.tensor.matmul(
                    yps[:, sb * 512:(sb + 1) * 512],
                    ot_acc[:, ci, qt * 128:(qt + 1) * 128],
                    wp[:, ci, sb * 512:(sb + 1) * 512],
                    start=(ci == 0),
                    stop=False,
                )
            nc.tensor.matmul(
                yps[:, sb * 512:(sb + 1) * 512],
                ones1,
                bp[:, sb * 512:(sb + 1) * 512],
                start=False,
                stop=True,
            )
        yd = ydp.tile([128, 1024], F32, tag="yd", name="yd")
        nc.vector.tensor_copy(yd, yps)
        nc.sync.dma_start(out=y[qt * 128:(qt + 1) * 128, :], in_=yd)
    for pool in (psD, ydp, wpp):
        pool.release()


def build_nc(lam: float):
    nc = bacc_mod.Bacc(None, target_bir_lowering=False)

    xbt = nc.declare_dram_parameter("xbt", [DIM, N], BF16, isOutput=False)
    wqkvt = nc.declare_dram_parameter("wqkvt", [DIM, 3 * DIM], BF16, isOutput=False)
    wprojt = nc.declare_dram_parameter("wprojt", [DIM, DIM], BF16, isOutput=False)
    bproj = nc.declare_dram_parameter("bproj", [1, DIM], BF16, isOutput=False)
    weff = nc.declare_dram_parameter("weff", [VD, 1], BF16, isOutput=False)
    y = nc.declare_dram_parameter("y", [NQ, DIM], F32, isOutput=True)

    ostore = nc.dram_tensor("ostore", [H, 2 * VD, NQ], BF16)

    with nc.allow_low_precision(reason="bf16 kernel, tolerance 2e-2"), \
         TileContext(nc) as tc:
        constp = tc.alloc_tile_pool(name="const", bufs=1)
        ones1 = constp.tile([1, 128], BF16, name="ones1")
        nc.vector.memset(ones1, 1.0)
        ones64 = constp.tile([VD, 1], BF16, name="ones64")
        nc.vector.memset(ones64, 1.0)
        eps16 = constp.tile([16, 1], F32, name="eps16")
        nc.vector.memset(eps16, EPS)
        weff_t = constp.tile([VD, 1], F32, name="weff_t")
        weff_bf = constp.tile([VD, 1], BF16, name="weff_bf")
        nc.sync.dma_start(out=weff_bf, in_=weff[:, :])
        nc.vector.tensor_copy(weff_t, weff_bf)

        denp = tc.alloc_tile_pool(name="den_p", bufs=1)
        den1 = denp.tile([H, NQ], F32, name="den1")
        den2 = denp.tile([H, NQ], F32, name="den2")

        packp = tc.alloc_tile_pool(name="packs", bufs=1)
        qpack = packp.tile([128, CIN, NQ], BF16, name="qpack")
        kpack = packp.tile([128, CIN, N], BF16, name="kpack")
        vpack = packp.tile([128, KT, H, VD + 1], BF16, name="vpack")
        nc.vector.memset(vpack[:, :, :, VD:VD + 1], 1.0)

        _phase_a(nc, tc, xbt, wqkvt, qpack, kpack, vpack)

        _phase_b(nc, tc, qpack, kpack, vpack, ostore, den1, den2)
        packp.release()  # frees qkv packs before combine working set opens

        accp = tc.alloc_tile_pool(name="acc_p", bufs=1)
        ot_acc = accp.tile([128, CIN, NQ], BF16, name="ot_acc")

        _phase_c(nc, tc, lam, ostore, den1, den2, ot_acc, ones64, eps16, weff_t)

        _phase_d(nc, tc, wprojt, bproj, ot_acc, ones1, y)
        accp.release()
        denp.release()
        constp.release()
    nc.finalize()
    return nc


def prepare(x, w_qkv, w_proj, b_proj, lambda_q1, lambda_k1, lambda_q2,
            lambda_k2, sub_norm_w):
    """Build (cached) program + per-core input maps."""
    x = np.asarray(x, np.float32)
    lam = float(
        np.exp(np.sum(np.float64(lambda_q1) * np.float64(lambda_k1)))
        - np.exp(np.sum(np.float64(lambda_q2) * np.float64(lambda_k2)))
        + LAMBDA_INIT
    )
    wqkvt = np.ascontiguousarray(np.asarray(w_qkv, np.float32).T).astype(NPBF)
    wprojt = np.ascontiguousarray(np.asarray(w_proj, np.float32).T).astype(NPBF)
    bp = np.asarray(b_proj, np.float32).reshape(1, DIM).astype(NPBF)
    weff = (np.asarray(sub_norm_w, np.float32) * (1.0 - LAMBDA_INIT)) \
        .reshape(VD, 1).astype(NPBF)

    key = round(lam, 12)
    if key not in _CACHE:
        _CACHE[key] = build_nc(lam)
    nc = _CACHE[key]

    in_maps = []
    for c in range(NCORES):
        b, half = c // 2, c % 2
        xt = np.asarray(x[b].T)  # [DIM, N]
        if half == 1:  # this core's query rows first
            xt = np.concatenate([xt[:, NQ:], xt[:, :NQ]], axis=1)
        in_maps.append({
            "xbt": np.ascontiguousarray(xt).astype(NPBF),
            "wqkvt": wqkvt,
            "wprojt": wprojt,
            "bproj": bp,
            "weff": weff,
        })
    return nc, in_maps


def kernel(x, w_qkv, w_proj, b_proj, lambda_q1, lambda_k1, lambda_q2,
           lambda_k2, sub_norm_w):
    nc, in_maps = prepare(x, w_qkv, w_proj, b_proj, lambda_q1, lambda_k1,
                          lambda_q2, lambda_k2, sub_norm_w)
    res = run_bass_kernel_spmd(nc, in_maps, list(range(NCORES)))
    out = np.empty((B, N, DIM), np.float32)
    for c in range(NCORES):
        b, half = c // 2, c % 2
        out[b, half * NQ:(half + 1) * NQ, :] = res.results[c]["y"]
    return out


# revision 15
# speedup vs baseline: 1.6963x; 1.3577x over previous
"""DiffAttention TRN2 kernel v3: bf16 + full-row S stationaries for full PE clock.

Sharding: 8 cores = (batch b in 0..3) x (query-half). Each core: 2048 query
rows, recomputes K/V for its batch's full 4096 keys. Zero collectives.

Key insight (measured): the PE clock governor only grants the full 2.4GHz
when the systolic array is ~fully row-active. 32-row stationaries (head_dim
contraction) pin the PE at 1.2GHz no matter what. So S matmuls use
zero-padded [128,128] stationaries: two persistent staging tiles are zeroed
once; each (h,br)'s K block is DMA'd into rows 0-31 from a DRAM store; rows
32-127 stay zero and kill the unused rows of the moving operand.

Other structure:
  - all matmuls bf16; K/Q co-tiles drain to DRAM (kstore/qstore), V drains
    into an SBUF-resident vpack [128, kt, h, 65] with a ones column so
    softmax denominators fall out of the PV matmul (row 64 of O^T).
  - attention: S matmul -> exp on ACT (PSUM->SBUF bf16) -> PV accumulate.
  - deferred combine: O^T rows 0-63 spill to DRAM bf16, denominator rows
    collect in SBUF [16,2048] (via DMA; engine APs need 32-aligned partition
    bases); one batched row-math pass, broadcasts via gpsimd
    partition_broadcast, elementwise split across DVE/Pool.
"""

import sys

import numpy as np

for p in ("/opt/trn_rl_repo",):
    if p not in sys.path:
        sys.path.insert(0, p)

import ml_dtypes

import concourse.bass as bass
import concourse.bacc as bacc_mod
import concourse.mybir as mybir
from concourse.bass_utils import run_bass_kernel_spmd
from concourse.tile import TileContext

F32 = mybir.dt.float32
BF16 = mybir.dt.bfloat16
NPBF = ml_dtypes.bfloat16

B, N, DIM, H, HD = 4, 4096, 1024, 16, 32
VD = 2 * HD  # 64
NQ = 2048
NCORES = 8
LAMBDA_INIT = 0.2
EPS = 1e-5
SCALE = HD ** -0.5
KT = N // 128  # 32 key tiles
CIN = DIM // 128  # 8

_CACHE = {}


def _phase_a(nc, tc, xbt, wqkvt, kstore, qstore, vpack):
    """qkv projections: K/Q co-tiles to DRAM, V into SBUF vpack."""
    xbp = tc.alloc_tile_pool(name="xb_p", bufs=2)
    wvp = tc.alloc_tile_pool(name="wv_p", bufs=1)
    wqp = tc.alloc_tile_pool(name="wq_p", bufs=8)
    qksp = tc.alloc_tile_pool(name="qks_p", bufs=4)
    psqk = tc.alloc_tile_pool(name="psQK", bufs=4, space="PSUM")
    psv_p = tc.alloc_tile_pool(name="psV", bufs=4, space="PSUM")

    wv = wvp.tile([128, CIN, DIM], BF16, name="wv")
    nc.sync.dma_start(
        out=wv,
        in_=wqkvt[:, 2 * DIM:3 * DIM].rearrange("(t p) c -> p t c", p=128),
    )
    dr_i = 0
    for hf in range(2):
        tok0 = hf * 2048
        xb = xbp.tile([128, CIN, 2048], BF16, tag="xb", name="xb")
        nc.sync.dma_start(
            out=xb,
            in_=xbt[:, tok0:tok0 + 2048].rearrange("(t p) n -> p t n", p=128),
        )
        for which in range(2):  # 0 = Q, 1 = K
            if which == 0 and hf >= 1:
                continue
            wofs = which * DIM
            dst = qstore if which == 0 else kstore
            for co in range(CIN):
                for ch in range(4):
                    ps = psqk.tile([128, 512], F32, tag="qk", name="psqk")
                    for ci in range(CIN):
                        wt = wqp.tile([128, 128], BF16, tag="w", name="wt")
                        nc.sync.dma_start(
                            out=wt,
                            in_=wqkvt[ci * 128:(ci + 1) * 128,
                                      wofs + co * 128:wofs + (co + 1) * 128],
                        )
                        nc.tensor.matmul(
                            ps,
                            wt,
                            xb[:, ci, ch * 512:(ch + 1) * 512],
                            start=(ci == 0),
                            stop=(ci == CIN - 1),
                        )
                    qks = qksp.tile([128, 512], BF16, tag="qks", name="qks")
                    if dr_i % 2 == 0:
                        nc.vector.tensor_copy(qks, ps)
                    else:
                        nc.scalar.copy(qks, ps)
                    dr_i += 1
                    nc.sync.dma_start(
                        out=dst[co, :, tok0 + ch * 512:tok0 + (ch + 1) * 512],
                        in_=qks,
                    )
        for ktq in range(16):  # V: tokens on partitions
            kt = hf * 16 + ktq
            psv = [
                psv_p.tile([128, 256], F32, tag="v", name="psv")
                for _ in range(4)
            ]
            for ci in range(CIN):
                for cc in range(4):
                    nc.tensor.matmul(
                        psv[cc],
                        xb[:, ci, ktq * 128:(ktq + 1) * 128],
                        wv[:, ci, cc * 256:(cc + 1) * 256],
                        start=(ci == 0),
                        stop=(ci == CIN - 1),
                    )
            for cc in range(4):
                dsl = vpack[:, kt, 4 * cc:4 * cc + 4, 0:VD]
                if dr_i % 2 == 0:
                    nc.vector.tensor_copy(dsl, psv[cc])
                else:
                    nc.scalar.copy(dsl, psv[cc])
                dr_i += 1
    for pool in (psv_p, psqk, qksp, wqp, wvp, xbp):
        pool.release()


def _phase_b(nc, tc, kstore, qstore, vpack, ostore, den1, den2):
    """attention with zero-padded full-row S stationaries."""
    psS = tc.alloc_tile_pool(name="psS", bufs=2, space="PSUM")
    psO = tc.alloc_tile_pool(name="psO", bufs=2, space="PSUM")
    esp = tc.alloc_tile_pool(name="es_p", bufs=4)
    stp = tc.alloc_tile_pool(name="st_p", bufs=3)
    kzp = tc.alloc_tile_pool(name="kz_p", bufs=1)
    dstp = tc.alloc_tile_pool(name="dst_p", bufs=2)

    # two persistent zero-padded staging tile pairs; rows 32-127 are zeroed
    # exactly once and never rewritten (zero weights kill the moving
    # operand's rows 32-127; explicit zeros there too, so no NaN*0).
    kstz = [kzp.tile([128, N], BF16, tag=f"kz{i}", name="kstz")
            for i in range(2)]
    qstz = [kzp.tile([128, NQ], BF16, tag=f"qz{i}", name="qstz")
            for i in range(2)]
    for i in range(2):
        nc.vector.memset(kstz[i], 0.0)
        nc.vector.memset(qstz[i], 0.0)

    for h in range(H):
        for br in range(2):
            idx = 2 * h + br
            co = idx // 4
            o32 = 32 * (idx % 4)
            kz = kstz[idx % 2]
            qz = qstz[idx % 2]
            nc.sync.dma_start(out=kz[0:HD, :], in_=kstore[co, o32:o32 + HD, :])
            nc.sync.dma_start(out=qz[0:HD, :], in_=qstore[co, o32:o32 + HD, :])
            for qc in range(2):
                o_ps = psO.tile([VD + 1, 1024], F32, tag="o", name="ops")
                for kt in range(KT):
                    sps = psS.tile([128, 1024], F32, tag="s", name="sps")
                    for sb in range(2):
                        nc.tensor.matmul(
                            sps[:, sb * 512:(sb + 1) * 512],
                            kz[:, kt * 128:(kt + 1) * 128],
                            qz[:, qc * 1024 + sb * 512:
                               qc * 1024 + (sb + 1) * 512],
                            start=True, stop=True,
                        )
                    es = esp.tile([128, 1024], BF16, tag="es", name="es")
                    nc.scalar.activation(
                        es, sps, mybir.ActivationFunctionType.Exp, scale=SCALE,
                    )
                    for sb in range(2):
                        nc.tensor.matmul(
                            o_ps[:, sb * 512:(sb + 1) * 512],
                            vpack[:, kt, h, :],
                            es[:, sb * 512:(sb + 1) * 512],
                            start=(kt == 0),
                            stop=(kt == KT - 1),
                        )
                stage = stp.tile([VD, 1024], BF16, tag="st", name="stage")
                nc.vector.tensor_copy(stage, o_ps[0:VD, :])
                # engine APs need 32-aligned partition base: stage the
                # denominator row then DMA it into den[h] (DMA is free-form).
                den = den1 if br == 0 else den2
                dstage = dstp.tile([1, 1024], F32, tag="ds", name="dstage")
                nc.vector.tensor_copy(dstage, o_ps[VD:VD + 1, :])
                nc.sync.dma_start(
                    out=den[h:h + 1, qc * 1024:(qc + 1) * 1024], in_=dstage,
                )
                nc.sync.dma_start(
                    out=ostore[h, br * VD:(br + 1) * VD,
                               qc * 1024:(qc + 1) * 1024],
                    in_=stage,
                )
    for pool in (dstp, kzp, stp, esp, psO, psS):
        pool.release()


def _phase_c(nc, tc, lam, ostore, den1, den2, ot_acc, ones64, eps16, weff_t):
    """deferred combine: diff-attn + RMS norm, batched row math."""
    rowp = tc.alloc_tile_pool(name="row_p", bufs=1)
    odp = tc.alloc_tile_pool(name="od_p", bufs=1)
    ldp = tc.alloc_tile_pool(name="ld_p", bufs=4)
    bcp = tc.alloc_tile_pool(name="bc_p", bufs=4)
    trp = tc.alloc_tile_pool(name="tr_p", bufs=1)
    psC = tc.alloc_tile_pool(name="psC", bufs=1, space="PSUM")

    # batched row math 1: t = lam * s1 / s2, r1 = 1/s1
    r2 = rowp.tile([H, NQ], F32, tag="rt", bufs=2, name="r2")
    nc.vector.reciprocal(r2, den2)
    t_row = rowp.tile([H, NQ], F32, tag="rt", bufs=2, name="t_row")
    nc.vector.tensor_scalar_mul(t_row, den1, lam)
    nc.vector.tensor_mul(t_row, t_row, r2)
    t_bf = rowp.tile([H, NQ], BF16, tag="tb", name="t_bf")
    nc.vector.tensor_copy(t_bf, t_row)
    r1 = rowp.tile([H, NQ], F32, tag="r1", name="r1")
    nc.vector.reciprocal(r1, den1)
    msr = rowp.tile([H, NQ], F32, tag="ms", name="msr")

    od_tiles = []
    for h in range(H):
        o1t = ldp.tile([VD, NQ], BF16, tag="ld", name="o1t")
        nc.sync.dma_start(out=o1t, in_=ostore[h, 0:VD, :])
        o2t = ldp.tile([VD, NQ], BF16, tag="ld", name="o2t")
        nc.sync.dma_start(out=o2t, in_=ostore[h, VD:2 * VD, :])
        trow = trp.tile([1, NQ], BF16, tag="tr", bufs=2, name="trow")
        nc.sync.dma_start(out=trow, in_=t_bf[h:h + 1, :])
        t_bc = bcp.tile([VD, NQ], BF16, tag="bc", bufs=2, name="t_bc")
        nc.gpsimd.partition_broadcast(t_bc, trow)
        tmp = bcp.tile([VD, NQ], BF16, tag="tmp", bufs=2, name="tmp")
        nc.vector.tensor_mul(tmp, t_bc, o2t)
        od = odp.tile([VD, NQ], BF16, tag=f"od{h}", name="od")
        nc.gpsimd.tensor_sub(od, o1t, tmp)
        od_tiles.append(od)
        sq = bcp.tile([VD, NQ], BF16, tag="sq", bufs=1, name="sq")
        nc.gpsimd.tensor_mul(sq, od, od)
        msp = psC.tile([1, NQ], F32, tag="ms", name="msp")
        for c4 in range(4):
            nc.tensor.matmul(
                msp[:, c4 * 512:(c4 + 1) * 512],
                ones64,
                sq[:, c4 * 512:(c4 + 1) * 512],
                start=True, stop=True,
            )
        mstage = trp.tile([1, NQ], F32, tag="msst", bufs=1, name="mstage")
        nc.vector.tensor_copy(mstage, msp)
        nc.sync.dma_start(out=msr[h:h + 1, :], in_=mstage)

    # batched row math 2: rr = 1/(s1*sqrt(mean(od^2)/s1^2 + eps))
    a_row = rowp.tile([H, NQ], F32, tag="rt", bufs=2, name="a_row")
    nc.vector.tensor_mul(a_row, msr, r1)
    nc.vector.tensor_mul(a_row, a_row, r1)
    sd = rowp.tile([H, NQ], F32, tag="rt", bufs=2, name="sd")
    nc.scalar.activation(
        sd, a_row, mybir.ActivationFunctionType.Sqrt,
        bias=eps16, scale=1.0 / VD,
    )
    rsd = rowp.tile([H, NQ], F32, tag="rt", bufs=2, name="rsd")
    nc.vector.reciprocal(rsd, sd)
    rr = rowp.tile([H, NQ], F32, tag="rt", bufs=2, name="rr")
    nc.vector.tensor_mul(rr, rsd, r1)
    rr_bf = rowp.tile([H, NQ], BF16, tag="rb", name="rr_bf")
    nc.vector.tensor_copy(rr_bf, rr)

    for h in range(H):
        rrow = trp.tile([1, NQ], BF16, tag="tr", bufs=2, name="rrow")
        nc.sync.dma_start(out=rrow, in_=rr_bf[h:h + 1, :])
        rr_bc = bcp.tile([VD, NQ], BF16, tag="bc", bufs=2, name="rr_bc")
        nc.gpsimd.partition_broadcast(rr_bc, rrow)
        odn = bcp.tile([VD, NQ], BF16, tag="tmp", bufs=2, name="odn")
        if h % 2 == 0:
            nc.vector.tensor_mul(odn, od_tiles[h], rr_bc)
            nc.gpsimd.tensor_scalar_mul(
                ot_acc[0:VD, h // 2, :], odn, weff_t,
            )
        else:
            nc.gpsimd.tensor_mul(odn, od_tiles[h], rr_bc)
            nc.vector.tensor_scalar_mul(
                ot_acc[VD:2 * VD, h // 2, :], odn, weff_t,
            )
    for pool in (psC, trp, bcp, ldp, odp, rowp):
        pool.release()


def _phase_d(nc, tc, wprojt, bproj, ot_acc, ones1, y):
    """output projection + bias."""
    wpp = tc.alloc_tile_pool(name="wp_p", bufs=1)
    ydp = tc.alloc_tile_pool(name="yd_p", bufs=3)
    psD = tc.alloc_tile_pool(name="psD", bufs=2, space="PSUM")

    wp = wpp.tile([128, CIN, DIM], BF16, name="wp")
    nc.sync.dma_start(
        out=wp, in_=wprojt[:, :].rearrange("(t p) n -> p t n", p=128),
    )
    bp = wpp.tile([1, DIM], BF16, name="bp")
    nc.sync.dma_start(out=bp, in_=bproj[:, :])
    for qt in range(NQ // 128):
        yps = psD.tile([128, 1024], F32, tag="y", name="yps")
        for sb in range(2):
            for ci in range(CIN):
                nc.tensor.matmul(
                    yps[:, sb * 512:(sb + 1) * 512],
                    ot_acc[:, ci, qt * 128:(qt + 1) * 128],
                    wp[:, ci, sb * 512:(sb + 1) * 512],
                    start=(ci == 0),
                    stop=False,
                )
            nc.tensor.matmul(
                yps[:, sb * 512:(sb + 1) * 512],
                ones1,
                bp[:, sb * 512:(sb + 1) * 512],
                start=False,
                stop=True,
            )
        yd = ydp.tile([128, 1024], F32, tag="yd", name="yd")
        nc.vector.tensor_copy(yd, yps)
        nc.sync.dma_start(out=y[qt * 128:(qt + 1) * 128, :], in_=yd)
    for pool in (psD, ydp, wpp):
        pool.release()


def build_nc(lam: float):
    nc = bacc_mod.Bacc(None, target_bir_lowering=False)

    xbt = nc.declare_dram_parameter("xbt", [DIM, N], BF16, isOutput=False)
    wqkvt = nc.declare_dram_parameter("wqkvt", [DIM, 3 * DIM], BF16, isOutput=False)
    wprojt = nc.declare_dram_parameter("wprojt", [DIM, DIM], BF16, isOutput=False)
    bproj = nc.declare_dram_parameter("bproj", [1, DIM], BF16, isOutput=False)
    weff = nc.declare_dram_parameter("weff", [VD, 1], BF16, isOutput=False)
    y = nc.declare_dram_parameter("y", [NQ, DIM], F32, isOutput=True)

    ostore = nc.dram_tensor("ostore", [H, 2 * VD, NQ], BF16)
    kstore = nc.dram_tensor("kstore", [CIN, 128, N], BF16)
    qstore = nc.dram_tensor("qstore", [CIN, 128, NQ], BF16)

    with nc.allow_low_precision(reason="bf16 kernel, tolerance 2e-2"), \
         TileContext(nc) as tc:
        constp = tc.alloc_tile_pool(name="const", bufs=1)
        ones1 = constp.tile([1, 128], BF16, name="ones1")
        nc.vector.memset(ones1, 1.0)
        ones64 = constp.tile([VD, 1], BF16, name="ones64")
        nc.vector.memset(ones64, 1.0)
        eps16 = constp.tile([16, 1], F32, name="eps16")
        nc.vector.memset(eps16, EPS)
        weff_t = constp.tile([VD, 1], F32, name="weff_t")
        weff_bf = constp.tile([VD, 1], BF16, name="weff_bf")
        nc.sync.dma_start(out=weff_bf, in_=weff[:, :])
        nc.vector.tensor_copy(weff_t, weff_bf)

        denp = tc.alloc_tile_pool(name="den_p", bufs=1)
        den1 = denp.tile([H, NQ], F32, name="den1")
        den2 = denp.tile([H, NQ], F32, name="den2")

        packp = tc.alloc_tile_pool(name="packs", bufs=1)
        vpack = packp.tile([128, KT, H, VD + 1], BF16, name="vpack")
        nc.vector.memset(vpack[:, :, :, VD:VD + 1], 1.0)

        _phase_a(nc, tc, xbt, wqkvt, kstore, qstore, vpack)
        _phase_b(nc, tc, kstore, qstore, vpack, ostore, den1, den2)
        packp.release()  # frees vpack before combine working set opens

        accp = tc.alloc_tile_pool(name="acc_p", bufs=1)
        ot_acc = accp.tile([128, CIN, NQ], BF16, name="ot_acc")

        _phase_c(nc, tc, lam, ostore, den1, den2, ot_acc, ones64, eps16, weff_t)

        _phase_d(nc, tc, wprojt, bproj, ot_acc, ones1, y)
        accp.release()
        denp.release()
        constp.release()
    nc.finalize()
    return nc


def prepare(x, w_qkv, w_proj, b_proj, lambda_q1, lambda_k1, lambda_q2,
            lambda_k2, sub_norm_w):
    """Build (cached) program + per-core input maps."""
    x = np.asarray(x, np.float32)
    lam = float(
        np.exp(np.sum(np.float64(lambda_q1) * np.float64(lambda_k1)))
        - np.exp(np.sum(np.float64(lambda_q2) * np.float64(lambda_k2)))
        + LAMBDA_INIT
    )
    wqkvt = np.ascontiguousarray(np.asarray(w_qkv, np.float32).T).astype(NPBF)
    wprojt = np.ascontiguousarray(np.asarray(w_proj, np.float32).T).astype(NPBF)
    bp = np.asarray(b_proj, np.float32).reshape(1, DIM).astype(NPBF)
    weff = (np.asarray(sub_norm_w, np.float32) * (1.0 - LAMBDA_INIT)) \
        .reshape(VD, 1).astype(NPBF)

    key = round(lam, 12)
    if key not in _CACHE:
        _CACHE[key] = build_nc(lam)
    nc = _CACHE[key]

    in_maps = []
    for c in range(NCORES):
        b, half = c // 2, c % 2
        xt = np.asarray(x[b].T)  # [DIM, N]
        if half == 1:  # this core's query rows first
            xt = np.concatenate([xt[:, NQ:], xt[:, :NQ]], axis=1)
        in_maps.append({
            "xbt": np.ascontiguousarray(xt).astype(NPBF),
            "wqkvt": wqkvt,
            "wprojt": wprojt,
            "bproj": bp,
            "weff": weff,
        })
    return nc, in_maps


def kernel(x, w_qkv, w_proj, b_proj, lambda_q1, lambda_k1, lambda_q2,
           lambda_k2, sub_norm_w):
    nc, in_maps = prepare(x, w_qkv, w_proj, b_proj, lambda_q1, lambda_k1,
                          lambda_q2, lambda_k2, sub_norm_w)
    res = run_bass_kernel_spmd(nc, in_maps, list(range(NCORES)))
    out = np.empty((B, N, DIM), np.float32)
    for c in range(NCORES):
        b, half = c // 2, c % 2
        out[b, half * NQ:(half + 1) * NQ, :] = res.results[c]["y"]
    return out


# revision 17
# speedup vs baseline: 1.8202x; 1.0731x over previous
"""DiffAttention TRN2 kernel v3: bf16 + full-row S stationaries for full PE clock.

Sharding: 8 cores = (batch b in 0..3) x (query-half). Each core: 2048 query
rows, recomputes K/V for its batch's full 4096 keys. Zero collectives.

Key insight (measured): the PE clock governor only grants the full 2.4GHz
when the systolic array is ~fully row-active. 32-row stationaries (head_dim
contraction) pin the PE at 1.2GHz no matter what. So S matmuls use
zero-padded [128,128] stationaries: two persistent staging tiles are zeroed
once; each (h,br)'s K block is DMA'd into rows 0-31 from a DRAM store; rows
32-127 stay zero and kill the unused rows of the moving operand.

Other structure:
  - all matmuls bf16; K/Q co-tiles drain to DRAM (kstore/qstore), V drains
    into an SBUF-resident vpack [128, kt, h, 65] with a ones column so
    softmax denominators fall out of the PV matmul (row 64 of O^T).
  - attention: S matmul -> exp on ACT (PSUM->SBUF bf16) -> PV accumulate.
  - deferred combine: O^T rows 0-63 spill to DRAM bf16, denominator rows
    collect in SBUF [16,2048] (via DMA; engine APs need 32-aligned partition
    bases); one batched row-math pass, broadcasts via gpsimd
    partition_broadcast, elementwise split across DVE/Pool.
"""

import sys

import numpy as np

for p in ("/opt/trn_rl_repo",):
    if p not in sys.path:
        sys.path.insert(0, p)

import ml_dtypes

import concourse.bass as bass
import concourse.bacc as bacc_mod
import concourse.mybir as mybir
from concourse.bass_utils import run_bass_kernel_spmd
from concourse.tile import TileContext

F32 = mybir.dt.float32
BF16 = mybir.dt.bfloat16
NPBF = ml_dtypes.bfloat16

B, N, DIM, H, HD = 4, 4096, 1024, 16, 32
VD = 2 * HD  # 64
NQ = 2048
NCORES = 8
LAMBDA_INIT = 0.2
EPS = 1e-5
SCALE = HD ** -0.5
KT = N // 128  # 32 key tiles
CIN = DIM // 128  # 8

_CACHE = {}


def _phase_a(nc, tc, xbt, wqkvt, kstore, qstore, vpack):
    """qkv projections: K/Q co-tiles to DRAM, V into SBUF vpack."""
    xbp = tc.alloc_tile_pool(name="xb_p", bufs=2)
    wvp = tc.alloc_tile_pool(name="wv_p", bufs=1)
    wqp = tc.alloc_tile_pool(name="wq_p", bufs=8)
    qksp = tc.alloc_tile_pool(name="qks_p", bufs=4)
    psqk = tc.alloc_tile_pool(name="psQK", bufs=4, space="PSUM")
    psv_p = tc.alloc_tile_pool(name="psV", bufs=4, space="PSUM")

    wv = wvp.tile([128, CIN, DIM], BF16, name="wv")
    nc.sync.dma_start(
        out=wv,
        in_=wqkvt[:, 2 * DIM:3 * DIM].rearrange("(t p) c -> p t c", p=128),
    )
    dr_i = 0
    xbs = []
    for hf in range(2):
        xb = xbp.tile([128, CIN, 2048], BF16, tag="xb", name="xb")
        nc.sync.dma_start(
            out=xb,
            in_=xbt[:, hf * 2048:hf * 2048 + 2048].rearrange(
                "(t p) n -> p t n", p=128),
        )
        xbs.append(xb)
    # K (both halves) + Q (first half) per co, in B's consumption order
    for co in range(CIN):
        for which in (1, 0):  # K first, then Q
            wofs = which * DIM
            dst = qstore if which == 0 else kstore
            for hf in range(2):
                if which == 0 and hf >= 1:
                    continue
                tok0 = hf * 2048
                for ch in range(4):
                    ps = psqk.tile([128, 512], F32, tag="qk", name="psqk")
                    for ci in range(CIN):
                        wt = wqp.tile([128, 128], BF16, tag="w", name="wt")
                        nc.sync.dma_start(
                            out=wt,
                            in_=wqkvt[ci * 128:(ci + 1) * 128,
                                      wofs + co * 128:wofs + (co + 1) * 128],
                        )
                        nc.tensor.matmul(
                            ps,
                            wt,
                            xbs[hf][:, ci, ch * 512:(ch + 1) * 512],
                            start=(ci == 0),
                            stop=(ci == CIN - 1),
                        )
                    qks = qksp.tile([128, 512], BF16, tag="qks", name="qks")
                    if dr_i % 2 == 0:
                        nc.vector.tensor_copy(qks, ps)
                    else:
                        nc.scalar.copy(qks, ps)
                    dr_i += 1
                    nc.sync.dma_start(
                        out=dst[co, :, tok0 + ch * 512:tok0 + (ch + 1) * 512],
                        in_=qks,
                    )
    # V by cc chunk (cc covers heads 4cc..4cc+3), all kt per chunk
    for cc in range(4):
        for kt in range(KT):
            hf, ktq = kt // 16, kt % 16
            psv = psv_p.tile([128, 256], F32, tag="v", name="psv")
            for ci in range(CIN):
                nc.tensor.matmul(
                    psv,
                    xbs[hf][:, ci, ktq * 128:(ktq + 1) * 128],
                    wv[:, ci, cc * 256:(cc + 1) * 256],
                    start=(ci == 0),
                    stop=(ci == CIN - 1),
                )
            dsl = vpack[:, kt, 4 * cc:4 * cc + 4, 0:VD]
            if dr_i % 2 == 0:
                nc.vector.tensor_copy(dsl, psv)
            else:
                nc.scalar.copy(dsl, psv)
            dr_i += 1
    for pool in (psv_p, psqk, qksp, wqp, wvp, xbp):
        pool.release()


def _phase_b(nc, tc, kstore, qstore, vpack, ostore, den1, den2):
    """attention with zero-padded full-row S stationaries."""
    psS = tc.alloc_tile_pool(name="psS", bufs=2, space="PSUM")
    psO = tc.alloc_tile_pool(name="psO", bufs=2, space="PSUM")
    esp = tc.alloc_tile_pool(name="es_p", bufs=4)
    stp = tc.alloc_tile_pool(name="st_p", bufs=3)
    kzp = tc.alloc_tile_pool(name="kz_p", bufs=1)
    dstp = tc.alloc_tile_pool(name="dst_p", bufs=2)

    # two persistent zero-padded staging tile pairs; rows 32-127 are zeroed
    # exactly once and never rewritten (zero weights kill the moving
    # operand's rows 32-127; explicit zeros there too, so no NaN*0).
    kstz = [kzp.tile([128, N], BF16, tag=f"kz{i}", name="kstz")
            for i in range(2)]
    qstz = [kzp.tile([128, NQ], BF16, tag=f"qz{i}", name="qstz")
            for i in range(2)]
    for i in range(2):
        nc.vector.memset(kstz[i], 0.0)
        nc.vector.memset(qstz[i], 0.0)

    for h in range(H):
        for br in range(2):
            idx = 2 * h + br
            co = idx // 4
            o32 = 32 * (idx % 4)
            kz = kstz[idx % 2]
            qz = qstz[idx % 2]
            nc.sync.dma_start(out=kz[0:HD, :], in_=kstore[co, o32:o32 + HD, :])
            nc.sync.dma_start(out=qz[0:HD, :], in_=qstore[co, o32:o32 + HD, :])
            for qc in range(2):
                o_ps = psO.tile([VD + 1, 1024], F32, tag="o", name="ops")
                for kt in range(KT):
                    sps = psS.tile([128, 1024], F32, tag="s", name="sps")
                    for sb in range(2):
                        nc.tensor.matmul(
                            sps[:, sb * 512:(sb + 1) * 512],
                            kz[:, kt * 128:(kt + 1) * 128],
                            qz[:, qc * 1024 + sb * 512:
                               qc * 1024 + (sb + 1) * 512],
                            start=True, stop=True,
                        )
                    es = esp.tile([128, 1024], BF16, tag="es", name="es")
                    nc.scalar.activation(
                        es, sps, mybir.ActivationFunctionType.Exp, scale=SCALE,
                    )
                    for sb in range(2):
                        nc.tensor.matmul(
                            o_ps[:, sb * 512:(sb + 1) * 512],
                            vpack[:, kt, h, :],
                            es[:, sb * 512:(sb + 1) * 512],
                            start=(kt == 0),
                            stop=(kt == KT - 1),
                        )
                stage = stp.tile([VD, 1024], BF16, tag="st", name="stage")
                nc.vector.tensor_copy(stage, o_ps[0:VD, :])
                # engine APs need 32-aligned partition base: stage the
                # denominator row then DMA it into den[h] (DMA is free-form).
                den = den1 if br == 0 else den2
                dstage = dstp.tile([1, 1024], F32, tag="ds", name="dstage")
                nc.vector.tensor_copy(dstage, o_ps[VD:VD + 1, :])
                nc.sync.dma_start(
                    out=den[h:h + 1, qc * 1024:(qc + 1) * 1024], in_=dstage,
                )
                nc.sync.dma_start(
                    out=ostore[h, br * VD:(br + 1) * VD,
                               qc * 1024:(qc + 1) * 1024],
                    in_=stage,
                )
    for pool in (dstp, kzp, stp, esp, psO, psS):
        pool.release()


def _phase_c(nc, tc, lam, ostore, den1, den2, ot_acc, ones64, eps16, weff_t):  # noqa: C901
    """deferred combine: diff-attn + RMS norm, batched row math."""
    rowp = tc.alloc_tile_pool(name="row_p", bufs=1)
    odp = tc.alloc_tile_pool(name="od_p", bufs=1)
    ldp = tc.alloc_tile_pool(name="ld_p", bufs=4)
    bcp = tc.alloc_tile_pool(name="bc_p", bufs=4)
    trp = tc.alloc_tile_pool(name="tr_p", bufs=1)
    psC = tc.alloc_tile_pool(name="psC", bufs=1, space="PSUM")

    # batched row math 1: t = lam * s1 / s2, r1 = 1/s1
    r2 = rowp.tile([H, NQ], F32, tag="rt", bufs=2, name="r2")
    nc.vector.reciprocal(r2, den2)
    t_row = rowp.tile([H, NQ], F32, tag="rt", bufs=2, name="t_row")
    nc.vector.tensor_scalar_mul(t_row, den1, lam)
    nc.vector.tensor_mul(t_row, t_row, r2)
    t_bf = rowp.tile([H, NQ], BF16, tag="tb", name="t_bf")
    nc.vector.tensor_copy(t_bf, t_row)
    r1 = rowp.tile([H, NQ], F32, tag="r1", name="r1")
    nc.vector.reciprocal(r1, den1)
    msr = rowp.tile([H, NQ], F32, tag="ms", name="msr")

    od_tiles = []
    for h in range(H):
        o1t = ldp.tile([VD, NQ], BF16, tag="ld", name="o1t")
        nc.sync.dma_start(out=o1t, in_=ostore[h, 0:VD, :])
        o2t = ldp.tile([VD, NQ], BF16, tag="ld", name="o2t")
        nc.sync.dma_start(out=o2t, in_=ostore[h, VD:2 * VD, :])
        trow = trp.tile([1, NQ], BF16, tag="tr", bufs=2, name="trow")
        nc.sync.dma_start(out=trow, in_=t_bf[h:h + 1, :])
        t_bc = bcp.tile([VD, NQ], BF16, tag="bc", bufs=2, name="t_bc")
        nc.gpsimd.partition_broadcast(t_bc, trow)
        tmp = bcp.tile([VD, NQ], BF16, tag="tmp", bufs=2, name="tmp")
        nc.vector.tensor_mul(tmp, t_bc, o2t)
        od = odp.tile([VD, NQ], BF16, tag=f"od{h}", name="od")
        nc.gpsimd.tensor_sub(od, o1t, tmp)
        od_tiles.append(od)
        sq = bcp.tile([VD, NQ], BF16, tag="sq", bufs=1, name="sq")
        nc.vector.tensor_mul(sq, od, od)
        msp = psC.tile([1, NQ], F32, tag="ms", name="msp")
        for c4 in range(4):
            nc.tensor.matmul(
                msp[:, c4 * 512:(c4 + 1) * 512],
                ones64,
                sq[:, c4 * 512:(c4 + 1) * 512],
                start=True, stop=True,
            )
        mstage = trp.tile([1, NQ], F32, tag="msst", bufs=1, name="mstage")
        nc.vector.tensor_copy(mstage, msp)
        nc.sync.dma_start(out=msr[h:h + 1, :], in_=mstage)

    # batched row math 2: rr = 1/(s1*sqrt(mean(od^2)/s1^2 + eps))
    a_row = rowp.tile([H, NQ], F32, tag="rt", bufs=2, name="a_row")
    nc.vector.tensor_mul(a_row, msr, r1)
    nc.vector.tensor_mul(a_row, a_row, r1)
    sd = rowp.tile([H, NQ], F32, tag="rt", bufs=2, name="sd")
    nc.scalar.activation(
        sd, a_row, mybir.ActivationFunctionType.Sqrt,
        bias=eps16, scale=1.0 / VD,
    )
    rsd = rowp.tile([H, NQ], F32, tag="rt", bufs=2, name="rsd")
    nc.vector.reciprocal(rsd, sd)
    rr = rowp.tile([H, NQ], F32, tag="rt", bufs=2, name="rr")
    nc.vector.tensor_mul(rr, rsd, r1)
    rr_bf = rowp.tile([H, NQ], BF16, tag="rb", name="rr_bf")
    nc.vector.tensor_copy(rr_bf, rr)

    for h in range(H):
        rrow = trp.tile([1, NQ], BF16, tag="tr", bufs=2, name="rrow")
        nc.sync.dma_start(out=rrow, in_=rr_bf[h:h + 1, :])
        rr_bc = bcp.tile([VD, NQ], BF16, tag="bc", bufs=2, name="rr_bc")
        nc.gpsimd.partition_broadcast(rr_bc, rrow)
        odn = bcp.tile([VD, NQ], BF16, tag="tmp", bufs=2, name="odn")
        nc.gpsimd.tensor_mul(odn, od_tiles[h], rr_bc)
        p0 = (h % 2) * VD
        nc.vector.tensor_scalar_mul(
            ot_acc[p0:p0 + VD, h // 2, :], odn, weff_t,
        )
    for pool in (psC, trp, bcp, ldp, odp, rowp):
        pool.release()


def _phase_d(nc, tc, wprojt, bproj, ot_acc, ones1, y):
    """output projection + bias."""
    wpp = tc.alloc_tile_pool(name="wp_p", bufs=1)
    ydp = tc.alloc_tile_pool(name="yd_p", bufs=3)
    psD = tc.alloc_tile_pool(name="psD", bufs=2, space="PSUM")

    wp = wpp.tile([128, CIN, DIM], BF16, name="wp")
    nc.sync.dma_start(
        out=wp, in_=wprojt[:, :].rearrange("(t p) n -> p t n", p=128),
    )
    bp = wpp.tile([1, DIM], BF16, name="bp")
    nc.sync.dma_start(out=bp, in_=bproj[:, :])
    for qt in range(NQ // 128):
        yps = psD.tile([128, 1024], F32, tag="y", name="yps")
        for sb in range(2):
            for ci in range(CIN):
                nc.tensor.matmul(
                    yps[:, sb * 512:(sb + 1) * 512],
                    ot_acc[:, ci, qt * 128:(qt + 1) * 128],
                    wp[:, ci, sb * 512:(sb + 1) * 512],
                    start=(ci == 0),
                    stop=False,
                )
            nc.tensor.matmul(
                yps[:, sb * 512:(sb + 1) * 512],
                ones1,
                bp[:, sb * 512:(sb + 1) * 512],
                start=False,
                stop=True,
            )
        yd = ydp.tile([128, 1024], F32, tag="yd", name="yd")
        nc.vector.tensor_copy(yd, yps)
        nc.sync.dma_start(out=y[qt * 128:(qt + 1) * 128, :], in_=yd)
    for pool in (psD, ydp, wpp):
        pool.release()


def build_nc(lam: float):
    nc = bacc_mod.Bacc(None, target_bir_lowering=False)

    xbt = nc.declare_dram_parameter("xbt", [DIM, N], BF16, isOutput=False)
    wqkvt = nc.declare_dram_parameter("wqkvt", [DIM, 3 * DIM], BF16, isOutput=False)
    wprojt = nc.declare_dram_parameter("wprojt", [DIM, DIM], BF16, isOutput=False)
    bproj = nc.declare_dram_parameter("bproj", [1, DIM], BF16, isOutput=False)
    weff = nc.declare_dram_parameter("weff", [VD, 1], BF16, isOutput=False)
    y = nc.declare_dram_parameter("y", [NQ, DIM], F32, isOutput=True)

    ostore = nc.dram_tensor("ostore", [H, 2 * VD, NQ], BF16)
    kstore = nc.dram_tensor("kstore", [CIN, 128, N], BF16)
    qstore = nc.dram_tensor("qstore", [CIN, 128, NQ], BF16)

    with nc.allow_low_precision(reason="bf16 kernel, tolerance 2e-2"), \
         TileContext(nc) as tc:
        constp = tc.alloc_tile_pool(name="const", bufs=1)
        ones1 = constp.tile([1, 128], BF16, name="ones1")
        nc.vector.memset(ones1, 1.0)
        ones64 = constp.tile([VD, 1], BF16, name="ones64")
        nc.vector.memset(ones64, 1.0)
        eps16 = constp.tile([16, 1], F32, name="eps16")
        nc.vector.memset(eps16, EPS)
        weff_t = constp.tile([VD, 1], F32, name="weff_t")
        weff_bf = constp.tile([VD, 1], BF16, name="weff_bf")
        nc.sync.dma_start(out=weff_bf, in_=weff[:, :])
        nc.vector.tensor_copy(weff_t, weff_bf)

        denp = tc.alloc_tile_pool(name="den_p", bufs=1)
        den1 = denp.tile([H, NQ], F32, name="den1")
        den2 = denp.tile([H, NQ], F32, name="den2")

        packp = tc.alloc_tile_pool(name="packs", bufs=1)
        vpack = packp.tile([128, KT, H, VD + 1], BF16, name="vpack")
        nc.vector.memset(vpack[:, :, :, VD:VD + 1], 1.0)

        _phase_a(nc, tc, xbt, wqkvt, kstore, qstore, vpack)
        _phase_b(nc, tc, kstore, qstore, vpack, ostore, den1, den2)
        packp.release()  # frees vpack before combine working set opens

        accp = tc.alloc_tile_pool(name="acc_p", bufs=1)
        ot_acc = accp.tile([128, CIN, NQ], BF16, name="ot_acc")

        _phase_c(nc, tc, lam, ostore, den1, den2, ot_acc, ones64, eps16, weff_t)

        _phase_d(nc, tc, wprojt, bproj, ot_acc, ones1, y)
        accp.release()
        denp.release()
        constp.release()
    nc.finalize()
    return nc


def prepare(x, w_qkv, w_proj, b_proj, lambda_q1, lambda_k1, lambda_q2,
            lambda_k2, sub_norm_w):
    """Build (cached) program + per-core input maps."""
    x = np.asarray(x, np.float32)
    lam = float(
        np.exp(np.sum(np.float64(lambda_q1) * np.float64(lambda_k1)))
        - np.exp(np.sum(np.float64(lambda_q2) * np.float64(lambda_k2)))
        + LAMBDA_INIT
    )
    wqkvt = np.ascontiguousarray(np.asarray(w_qkv, np.float32).T).astype(NPBF)
    wprojt = np.ascontiguousarray(np.asarray(w_proj, np.float32).T).astype(NPBF)
    bp = np.asarray(b_proj, np.float32).reshape(1, DIM).astype(NPBF)
    weff = (np.asarray(sub_norm_w, np.float32) * (1.0 - LAMBDA_INIT)) \
        .reshape(VD, 1).astype(NPBF)

    key = round(lam, 12)
    if key not in _CACHE:
        _CACHE[key] = build_nc(lam)
    nc = _CACHE[key]

    in_maps = []
    for c in range(NCORES):
        b, half = c // 2, c % 2
        xt = np.asarray(x[b].T)  # [DIM, N]
        if half == 1:  # this core's query rows first
            xt = np.concatenate([xt[:, NQ:], xt[:, :NQ]], axis=1)
        in_maps.append({
            "xbt": np.ascontiguousarray(xt).astype(NPBF),
            "wqkvt": wqkvt,
            "wprojt": wprojt,
            "bproj": bp,
            "weff": weff,
        })
    return nc, in_maps


def kernel(x, w_qkv, w_proj, b_proj, lambda_q1, lambda_k1, lambda_q2,
           lambda_k2, sub_norm_w):
    nc, in_maps = prepare(x, w_qkv, w_proj, b_proj, lambda_q1, lambda_k1,
                          lambda_q2, lambda_k2, sub_norm_w)
    res = run_bass_kernel_spmd(nc, in_maps, list(range(NCORES)))
    out = np.empty((B, N, DIM), np.float32)
    for c in range(NCORES):
        b, half = c // 2, c % 2
        out[b, half * NQ:(half + 1) * NQ, :] = res.results[c]["y"]
    return out
